# revision 26
# baseline (speedup 1.0000x reference)
"""Trainium2 Bass kernel for nn_Attn_30820685316537 (segment_reduce attention).

Reference computation (per batch b):
    score = output @ context^T                     [Q, S]
    avg   = per-segment mean of score over S, broadcast back
    align = softmax(avg, axis=S)                   [Q, S]
    ac    = align @ context                        [Q, D]
    out   = tanh(concat(ac, output) @ W^T + bias)  [Q, D]
    returns (out, align)

Key algebraic structure exploited on device: `avg` is constant within each of
the 64 contiguous segments, so the whole pipeline factors through rank-64
segment space.  With Csum[n, d] = sum_{s in segment n} context[s, d]:
    segavg[q, n]  = (output[q, :] . Csum[n, :]) / max(cnt[n], 1)
    Enorm[q, n]   = softmax weights per segment (exp/sum with exact counts)
    align[q, s]   = Enorm[q, seg[s]]        (broadcast via 0/1 one-hot matmul)
    ac[q, d]      = sum_n Enorm[q, n] * Csum[n, d]
This removes both S-sized GEMMs while computing the exact same function
(bilinearity of the segment sum; fp reassociation only).

Sharding: data-parallel over batch B=16 across 8 NeuronCores (2 batches per
core); W replicated.  Matmuls run as float32r (full-rate fp32 mode of the PE,
~1e-4 class rel err); the softmax middle section stays in exact fp32.

Emission order software-pipelines the two batches (front(0), front(1),
back(0), back(1)) so the PE always has independent work during each batch's
serial softmax chain; input DMAs ride the Sync HWDGE ring, output DMAs the
Scalar ring to avoid head-of-line blocking.
"""
import numpy as np
from contextlib import ExitStack

B, Q, S, D = 16, 512, 1024, 1024
NSEG = 64
NCORES = 8
BPC = B // NCORES          # batches per core
QT = Q // 128              # 4 q-tiles
ST = S // 128              # 8 s-tiles
DT = D // 128              # 8 d-tiles
FT = 2 * D // 128          # 16 f-tiles of W^T

_CACHE = {}


def _build_nc():
    import concourse.bacc as bacc
    import concourse.tile as tile
    import concourse.mybir as mybir

    f32 = mybir.dt.float32
    f32r = mybir.dt.float32r
    f16 = mybir.dt.float16

    nc = bacc.Bacc("TRN2", target_bir_lowering=False, debug=False,
                   enable_asserts=False, num_devices=NCORES)

    ot_in = nc.dram_tensor("ot_in", [BPC, D, Q], f32r, kind="ExternalInput")  # output^T
    context_in = nc.dram_tensor("context_in", [BPC, S, D], f32r, kind="ExternalInput")
    wt_in = nc.dram_tensor("wt_in", [2, 2 * D, D // 2], f16, kind="ExternalInput")  # W^T e-halves, fp16
    biasr_in = nc.dram_tensor("biasr_in", [1, D], f16, kind="ExternalInput")    # bias row fp16
    ones_in = nc.dram_tensor("ones_in", [1, 128], f16, kind="ExternalInput")
    ident_in = nc.dram_tensor("ident_in", [128, 128], f32, kind="ExternalInput")
    identr_in = nc.dram_tensor("identr_in", [128, 128], f32r, kind="ExternalInput")
    oh_in = nc.dram_tensor("oh_in", [BPC, 128, ST * NSEG], f32r, kind="ExternalInput")
    invc_in = nc.dram_tensor("invc_in", [BPC, NSEG, 1], f32, kind="ExternalInput")
    cntb_in = nc.dram_tensor("cntb_in", [BPC, 128, QT * NSEG], f32, kind="ExternalInput")

    out_o = nc.dram_tensor("out_o", [BPC, Q, D], f32, kind="ExternalOutput")
    align_o = nc.dram_tensor("align_o", [BPC, Q, S], f32, kind="ExternalOutput")

    Exp = mybir.ActivationFunctionType.Exp
    Tanh = mybir.ActivationFunctionType.Tanh

    with tile.TileContext(nc) as tc, ExitStack() as ctx:
        consts = ctx.enter_context(tc.tile_pool(name="consts", bufs=1))
        wt_pool = ctx.enter_context(tc.tile_pool(name="wt", bufs=1))
        aux = ctx.enter_context(tc.tile_pool(name="aux", bufs=2))
        c_pool = ctx.enter_context(tc.tile_pool(name="cp", bufs=7))
        ot_pool = ctx.enter_context(tc.tile_pool(name="otp", bufs=2))
        act_pool = ctx.enter_context(tc.tile_pool(name="actp", bufs=2))
        mid = ctx.enter_context(tc.tile_pool(name="mid", bufs=2))      # live into back()
        mid1 = ctx.enter_context(tc.tile_pool(name="mid1", bufs=1))    # front-transient
        stage = ctx.enter_context(tc.tile_pool(name="stage", bufs=2))

        ps_cs = ctx.enter_context(tc.tile_pool(name="ps_cs", bufs=2, space="PSUM"))
        ps_sm = ctx.enter_context(tc.tile_pool(name="ps_sm", bufs=2, space="PSUM"))
        ps_mm = ctx.enter_context(tc.tile_pool(name="ps_mm", bufs=4, space="PSUM"))

        ident = consts.tile([128, 128], f32, tag="ident")
        nc.sync.dma_start(ident[:], ident_in.ap())
        identr = consts.tile([128, 128], f32r, tag="identr")
        nc.sync.dma_start(identr[:], identr_in.ap())
        biasr_sb = consts.tile([1, D], f16, tag="biasr")
        ones_sb = consts.tile([1, 128], f16, tag="ones")
        wt_sb = []
        state = [dict() for _ in range(BPC)]

        def emit_loads_aux(b, eng):
            st = state[b]
            oh = aux.tile([128, ST * NSEG], f32r, tag="oh")
            eng.dma_start(oh[:], oh_in.ap()[b])
            invc = aux.tile([NSEG, 1], f32, tag="invc")
            eng.dma_start(invc[:], invc_in.ap()[b])
            cntb = aux.tile([128, QT * NSEG], f32, tag="cntb")
            eng.dma_start(cntb[:], cntb_in.ap()[b])
            st["oh"], st["invc"], st["cntb"] = oh, invc, cntb

        def emit_loads_c(b, eng):
            st = state[b]
            c_sb = []
            for i in range(ST):
                c_i = c_pool.tile([128, D], f32r, tag="c")
                eng.dma_start(c_i[:], context_in.ap()[b, 128 * i:128 * (i + 1), :])
                c_sb.append(c_i)
            st["c"] = c_sb

        def emit_loads_ot(b, eng):
            st = state[b]
            ot_sb = []
            for k in range(DT):
                otk = ot_pool.tile([128, Q], f32r, tag=f"ot{k}")
                eng.dma_start(otk[:], ot_in.ap()[b, 128 * k:128 * (k + 1), :])
                ot_sb.append(otk)
            st["ot"] = ot_sb

        def emit_ohT(b):
            # ohT [64, S] from oh on-device: 8 PE transposes of [128s, 64] + 2 copies
            st = state[b]
            oh = st["oh"]
            ohT = aux.tile([NSEG, S], f32r, tag="ohT")
            for g in range(2):
                po = ps_sm.tile([NSEG, 512], f32r, tag="pss")
                for h in range(4):
                    i = 4 * g + h
                    nc.tensor.transpose(po[0:NSEG, 128 * h:128 * (h + 1)],
                                        oh[:, NSEG * i:NSEG * (i + 1)], identr[:])
                nc.vector.tensor_copy(ohT[:, 512 * g:512 * (g + 1)], po[:])
            st["ohT"] = ohT

        def emit_front(b):
            st = state[b]
            oh, ohT, invc, cntb = st["oh"], st["ohT"], st["invc"], st["cntb"]
            ot_sb = st["ot"]

            # Csum[n, d] = sum_{s in seg n} C[s, d]  (2 psum halves)
            cs0 = ps_cs.tile([NSEG, 512], f32, tag="cs")
            cs1 = ps_cs.tile([NSEG, 512], f32, tag="cs")
            for i in range(ST):
                c_i = st["c"][i]
                nc.tensor.matmul(cs0[:], oh[:, NSEG * i:NSEG * (i + 1)],
                                 c_i[:, 0:512], start=(i == 0), stop=(i == ST - 1))
                nc.tensor.matmul(cs1[:], oh[:, NSEG * i:NSEG * (i + 1)],
                                 c_i[:, 512:1024], start=(i == 0), stop=(i == ST - 1))
            csum_sb = mid.tile([NSEG, D], f32r, tag="csum")
            nc.vector.tensor_copy(csum_sb[:, 0:512], cs0[:])
            nc.vector.tensor_copy(csum_sb[:, 512:1024], cs1[:])
            st["csum"] = csum_sb

            # CsumT: 8 transposes of [64,128] -> packed [128, 64*8]
            csumT_sb = mid1.tile([128, NSEG * DT], f32r, tag="csumT")
            for g in range(2):
                pss = ps_sm.tile([128, 256], f32r, tag="pss")
                for h in range(4):
                    d = 4 * g + h
                    nc.tensor.transpose(pss[:, 64 * h:64 * (h + 1)],
                                        csum_sb[0:NSEG, 128 * d:128 * (d + 1)],
                                        identr[0:NSEG, 0:NSEG])
                nc.vector.tensor_copy(csumT_sb[:, 256 * g:256 * (g + 1)], pss[:])

            # segavgT[n, q] = (Csum @ O^T)[n, q] * invc[n]
            sg = ps_cs.tile([NSEG, Q], f32, tag="cs")
            for d in range(DT):
                nc.tensor.matmul(sg[:], csumT_sb[:, NSEG * d:NSEG * (d + 1)],
                                 ot_sb[d][:], start=(d == 0), stop=(d == DT - 1))
            segavgT_sb = mid1.tile([NSEG, Q], f32, tag="segavgT")
            nc.vector.tensor_scalar_mul(segavgT_sb[:], sg[:], invc[:])

            # segavg [q, (j n)] via 4 transposes (exact fp32)
            pss2 = ps_sm.tile([128, QT * NSEG], f32, tag="pss")
            for j in range(QT):
                nc.tensor.transpose(pss2[:, NSEG * j:NSEG * (j + 1)],
                                    segavgT_sb[0:NSEG, 128 * j:128 * (j + 1)],
                                    ident[0:NSEG, 0:NSEG])
            segavg_sb = mid1.tile([128, QT * NSEG], f32, tag="segavg")
            nc.vector.tensor_copy(segavg_sb[:], pss2[:])

            # softmax over segments with exact counts
            mx = mid1.tile([128, QT], f32, tag="mx")
            nc.vector.reduce_max(mx[:], segavg_sb[:].rearrange("p (j n) -> p j n", n=NSEG),
                                 axis=mybir.AxisListType.X)
            neg_mx = mid1.tile([128, QT], f32, tag="neg_mx")
            nc.vector.tensor_scalar_mul(neg_mx[:], mx[:], -1.0)
            e_sb = mid1.tile([128, QT * NSEG], f32, tag="e")
            for j in range(QT):
                nc.scalar.activation(e_sb[:, NSEG * j:NSEG * (j + 1)],
                                     segavg_sb[:, NSEG * j:NSEG * (j + 1)],
                                     Exp, bias=neg_mx[:, j:j + 1])
            w_sb = mid1.tile([128, QT * NSEG], f32, tag="w")
            nc.vector.tensor_mul(w_sb[:], e_sb[:], cntb[:])
            dsum = mid1.tile([128, QT], f32, tag="dsum")
            nc.vector.reduce_sum(dsum[:], w_sb[:].rearrange("p (j n) -> p j n", n=NSEG),
                                 axis=mybir.AxisListType.X)
            rd = mid1.tile([128, QT], f32, tag="rd")
            nc.vector.reciprocal(rd[:], dsum[:])
            enorm_sb = mid1.tile([128, QT * NSEG], f32, tag="enorm")
            for j in range(QT):
                nc.vector.tensor_scalar_mul(enorm_sb[:, NSEG * j:NSEG * (j + 1)],
                                            e_sb[:, NSEG * j:NSEG * (j + 1)],
                                            rd[:, j:j + 1])

            # EnormT [n, q] via 4 transposes, converted to f32r
            pse = ps_sm.tile([NSEG, Q], f32, tag="pss")
            for j in range(QT):
                nc.tensor.transpose(pse[0:NSEG, 128 * j:128 * (j + 1)],
                                    enorm_sb[:, NSEG * j:NSEG * (j + 1)], ident[:])
            enT_sb = mid.tile([NSEG, Q], f32r, tag="enT")
            nc.vector.tensor_copy(enT_sb[:], pse[:])
            st["enT"] = enT_sb

        def emit_mid(b):
            st = state[b]
            ohT, csum_sb, enT_sb = st["ohT"], st["csum"], st["enT"]

            # align output: Enorm broadcast through one-hot^T
            for j in range(QT):
                for h in range(2):
                    pa = ps_mm.tile([128, 512], f32, tag="pmm")
                    nc.tensor.matmul(pa[:], enT_sb[0:NSEG, 128 * j:128 * (j + 1)],
                                     ohT[:, 512 * h:512 * (h + 1)],
                                     start=True, stop=True)
                    stg = stage.tile([128, 512], f32, tag="al_st")
                    nc.vector.tensor_copy(stg[:], pa[:])
                    nc.scalar.dma_start(
                        align_o.ap()[b, 128 * j:128 * (j + 1), 512 * h:512 * (h + 1)],
                        stg[:])

            # aligned-context^T tiles: ACT_d[dd, q] = sum_n Csum[n, dd]·EnormT[n, q]
            act_sb = []
            for d in range(DT):
                pm = ps_mm.tile([128, 512], f32, tag="pmm")
                nc.tensor.matmul(pm[:], csum_sb[0:NSEG, 128 * d:128 * (d + 1)],
                                 enT_sb[:], start=True, stop=True)
                actd = act_pool.tile([128, Q], f16, tag=f"act{d}")
                nc.vector.tensor_copy(actd[:], pm[:])
                act_sb.append(actd)
            st["act"] = act_sb

            # fp16 copy of O^T for the fp16 M3 pass
            ot16_sb = []
            for k in range(DT):
                o16 = act_pool.tile([128, Q], f16, tag=f"ot16_{k}")
                nc.vector.tensor_copy(o16[:], st["ot"][k][:])
                ot16_sb.append(o16)
            st["ot16"] = ot16_sb

        def emit_m3(b, eb, wt_map):
            st = state[b]
            act_sb, ot_sb = st["act"], st["ot16"]
            for j in range(QT):
                pm = ps_mm.tile([128, 512], f32, tag="pmm")
                nc.tensor.matmul(pm[:], ones_sb[:],
                                 biasr_sb[:, 512 * eb:512 * (eb + 1)],
                                 start=True, stop=False)
                for f in range(FT):
                    lhs = (act_sb[f] if f < DT else ot_sb[f - DT])
                    nc.tensor.matmul(pm[:], lhs[:, 128 * j:128 * (j + 1)],
                                     wt_map[(eb, f)][:],
                                     start=False, stop=(f == FT - 1))
                ost = stage.tile([128, 512], f32, tag="out_st")
                nc.scalar.activation(ost[:], pm[:], Tanh)
                nc.scalar.dma_start(
                    out_o.ap()[b, 128 * j:128 * (j + 1), 512 * eb:512 * (eb + 1)],
                    ost[:])

        def emit_wt(eb):
            for f in range(FT):
                w = wt_pool.tile([128, D // 2], f16, tag=f"wt{eb}_{f}")
                nc.sync.dma_start(w[:], wt_in.ap()[eb, 128 * f:128 * (f + 1), :])
                wt_map[(eb, f)] = w

        wt_map = {}
        # Ring plan: Sync = aux0, C0, WTe0, OT1, WTe1; Scalar = OT0 + outputs;
        # GpSimd = aux1 + slot-gated C1.
        emit_loads_aux(0, nc.sync)
        emit_loads_ot(0, nc.scalar)
        emit_loads_c(0, nc.sync)
        emit_loads_aux(1, nc.gpsimd)
        emit_loads_c(1, nc.gpsimd)
        emit_wt(0)
        emit_wt(1)
        emit_loads_ot(1, nc.scalar)
        nc.sync.dma_start(biasr_sb[:], biasr_in.ap())
        nc.sync.dma_start(ones_sb[:], ones_in.ap())

        emit_ohT(0)
        emit_front(0)
        emit_mid(0)
        emit_m3(0, 0, wt_map)
        emit_ohT(1)
        emit_front(1)
        emit_mid(1)
        emit_m3(0, 1, wt_map)
        emit_m3(1, 0, wt_map)
        emit_m3(1, 1, wt_map)

    nc.compile()
    return nc


def _host_prep(output, context, W_weight, W_bias, segment_ids):
    """Shard over batch + build per-core input maps (host-side index prep)."""
    wt_full = W_weight.T.astype(np.float16)                            # [2D, D] fp16
    wt = np.ascontiguousarray(
        np.stack([wt_full[:, :D // 2], wt_full[:, D // 2:]]))          # [2, 2D, D/2]
    biasr = np.ascontiguousarray(W_bias.astype(np.float16)[None, :])
    ones = np.ones((1, 128), dtype=np.float16)
    ident = np.eye(128, dtype=np.float32)

    in_maps = []
    for c in range(NCORES):
        lo = c * BPC
        ohs, invcs, cntbs = [], [], []
        for b in range(BPC):
            ids = segment_ids[lo + b].astype(np.int32)                # [S]
            oh = (ids[:, None] == np.arange(NSEG, dtype=np.int32)[None, :]
                  ).astype(np.float32)                                # [S, NSEG]
            cnt = oh.sum(axis=0)                                      # [NSEG]
            inv = (1.0 / np.maximum(cnt, 1.0)).astype(np.float32)
            oh_packed = np.ascontiguousarray(
                oh.reshape(ST, 128, NSEG).transpose(1, 0, 2).reshape(128, ST * NSEG))
            ohs.append(oh_packed)
            invcs.append(inv[:, None])
            cntbs.append(np.ascontiguousarray(
                np.broadcast_to(np.tile(cnt.astype(np.float32), QT)[None, :],
                                (128, QT * NSEG))))
        in_maps.append({
            "ot_in": np.ascontiguousarray(
                output[lo:lo + BPC].astype(np.float32).transpose(0, 2, 1)),
            "context_in": np.ascontiguousarray(context[lo:lo + BPC].astype(np.float32)),
            "wt_in": wt,
            "biasr_in": biasr,
            "ones_in": ones,
            "ident_in": ident,
            "identr_in": ident,
            "oh_in": np.stack(ohs),
            "invc_in": np.stack(invcs),
            "cntb_in": np.stack(cntbs),
        })
    return in_maps


def _run(inputs, trace=False, tmpdir=None):
    from concourse.bass_utils import run_bass_kernel_spmd
    if "nc" not in _CACHE:
        _CACHE["nc"] = _build_nc()
    nc = _CACHE["nc"]
    in_maps = _host_prep(**inputs)
    kw = {}
    if trace:
        kw = {"trace": True, "tmpdir": tmpdir}
    res = run_bass_kernel_spmd(nc, in_maps, core_ids=list(range(NCORES)), **kw)
    out = np.concatenate([res.results[c]["out_o"] for c in range(NCORES)], axis=0)
    align = np.concatenate([res.results[c]["align_o"] for c in range(NCORES)], axis=0)
    return (out, align), res


def kernel(output, context, W_weight, W_bias, segment_ids):
    (out, align), _ = _run(dict(output=output, context=context, W_weight=W_weight,
                                W_bias=W_bias, segment_ids=segment_ids))
    return out, align
